# revision 26
# baseline (speedup 1.0000x reference)
"""Multi-head attention (B=2, T=2048, D=1024, H=16, no causal mask) on 8 trn2
NeuronCores.

Sharding: pure data-parallel over (batch, query-token-block).  Core c handles
batch b = c // 4 and query rows [tb*512, (tb+1)*512) with tb = c % 4.  Each
core redundantly computes K and V for its whole batch (15.1 GFLOP/core vs 8.6
for tensor-parallel-heads) but needs NO collectives; an on-chip 4-rank 8 MB
AllReduce costs more than the redundant compute.

Precision: x/Wqkv and the whole QKV+attention pipeline run in fp16 (PE at
1 cyc/row with fp32 PSUM accumulation; fp32r measures ~2 cyc/row on HW even
warm; fp16 keeps 3 more mantissa bits than bf16 -> ~6e-4 final rel err vs
5e-3).  The output projection — whose input rounding feeds the result
directly — stays in float32r.

Per-core plan:
  1. PE-transpose X[b] (bf16) into XT (1024x2048 SBUF) and the query slice
     Xq into XqT.
  2. QT[do,:] = Wq[:,do]^T @ XqT       (QT:  [1024, 512]  bf16 SBUF)
     KT[do,:] = Wk[:,do]^T @ XT        (KT:  [1024, 2048] bf16 SBUF)
     V [tc,:] = XT[:,tc]^T @ Wv        (V:   [2048, 1024] bf16 SBUF, 65-wide
                                        head slots with a ones column ->
                                        PV also yields the softmax sum)
  3. Attention per head pair p (heads 2p, 2p+1 at partitions 0-63 / 64-127
     of KT/QT row chunk p):
       logitsT[k,q] = KT_h[:,kc]^T @ QT_h      (PSUM [128, 512] fp32)
       PT = exp(0.125 * logitsT)               (ScalarE, PSUM -> bf16 SBUF)
       outT_h[dh,q], s[q] = [V_h | 1]^T @ PT   (PSUM [65, 512], accum 16 kc)
     normalize: outT_h *= (1/s) broadcast across partitions via DMA,
     written to ot_sb as float32r.
  4. y[q,:] = outT^T @ Wo in float32r (accumulate 8 row chunks).
"""

import numpy as np

import concourse.bacc as bacc
import concourse.mybir as mybir
import concourse.tile as tile

F32 = mybir.dt.float32
F32R = mybir.dt.float32r
F16 = mybir.dt.float16

B, T, D, H = 2, 2048, 1024, 16
DH = D // H  # 64
TQ = 512     # query tokens per core
N_CORES = 8
P = 128
KD = D // P        # 8 contraction chunks over D
NT = T // P        # 16 key-token chunks
NTB = T // TQ      # 4 token blocks
NPAIR = H // 2     # 8 head pairs
VW = DH + 1        # 65: V head slot width incl. ones column
NQ = TQ // P       # 4 query-token chunks
EXPF = mybir.ActivationFunctionType.Exp


def build_nc():
    nc = bacc.Bacc("TRN2", target_bir_lowering=False, debug=False,
                   num_devices=N_CORES)
    xb = nc.dram_tensor("xb", [T, D], F16, kind="ExternalInput").ap()
    xq = nc.dram_tensor("xq", [TQ, D], F16, kind="ExternalInput").ap()
    wqkv = nc.dram_tensor("wqkv", [D, 3 * D], F16, kind="ExternalInput").ap()
    wo = nc.dram_tensor("wo", [D, D], F16, kind="ExternalInput").ap()
    y = nc.dram_tensor("y", [TQ, D], F32, kind="ExternalOutput").ap()

    with tile.TileContext(nc) as tc:
        with tc.tile_pool(name="persist", bufs=1) as persist:
            v_sb = persist.tile([P, NT * H * VW], F16)    # 32.5 KB/part
            qt_sb = persist.tile([P, NPAIR * TQ], F16)    # 8 KB/part
            kt_sb = persist.tile([P, KD * T], F16)        # 32 KB/part
            # ones columns in every (tok-chunk, head) V slot
            onec = persist.tile([P, 1], F16)
            nc.vector.memset(onec[:], 1.0)
            nc.vector.tensor_copy(
                v_sb.rearrange("p (b c) -> p b c", c=VW)[:, :, DH:DH + 1],
                onec.unsqueeze(1).broadcast_to((P, NT * H, 1)))

            # ---------------- phase A-D: transposes + projections ----------
            with (
                tc.tile_pool(name="xtp", bufs=1) as xtp,
                tc.tile_pool(name="wp", bufs=1) as wp,
                tc.tile_pool(name="pjps", bufs=5, space="PSUM") as pjps,
            ):
                xt = xtp.tile([P, KD * T], F16)    # 32 KB/part
                xqt = xtp.tile([P, KD * TQ], F16)  # 8 KB/part

                # A: fp16 is 2 bytes -> the DMA xbar can transpose straight
                # from DRAM into SBUF; no PE transposes, no psum, no copies.
                for kd in range(KD):
                    nc.sync.dma_start_transpose(
                        xqt[:, kd * TQ:(kd + 1) * TQ],
                        xq[:, kd * P:(kd + 1) * P])
                    nc.sync.dma_start_transpose(
                        xt[:, kd * T:(kd + 1) * T],
                        xb[:, kd * P:(kd + 1) * P])

                # wqkv viewed as [p, ko, col] so a whole K-column strip of a
                # weight loads with one DMA
                wq3 = wqkv.rearrange("(ko p) c -> p ko c", p=P)

                # D: QT (dout chunk do covers heads 2do, 2do+1)
                for do in range(KD):
                    wt = wp.tile([P, KD * P], F16, tag="wk", bufs=2)
                    nc.sync.dma_start(
                        wt.rearrange("p (ko c) -> p ko c", c=P),
                        wq3[:, :, do * P:(do + 1) * P])
                    pq = pjps.tile([P, TQ], F32, tag="pj")
                    for kd in range(KD):
                        nc.tensor.matmul(
                            pq[:], wt[:, kd * P:(kd + 1) * P],
                            xqt[:, kd * TQ:(kd + 1) * TQ],
                            start=(kd == 0), stop=(kd == KD - 1))
                    nc.any.tensor_copy(qt_sb[:, do * TQ:(do + 1) * TQ], pq[:])

                # B: KT.  kd-outer / tb-inner: one LDWEIGHTS serves 4 matmuls
                # (4 open psum accumulators).
                for do in range(KD):
                    wt = wp.tile([P, KD * P], F16, tag="wk", bufs=2)
                    nc.sync.dma_start(
                        wt.rearrange("p (ko c) -> p ko c", c=P),
                        wq3[:, :, D + do * P: D + (do + 1) * P])
                    pks = [pjps.tile([P, TQ], F32, tag="pj", name=f"pk{do}_{_t}")
                           for _t in range(NTB)]
                    for kd in range(KD):
                        for tb in range(NTB):
                            nc.tensor.matmul(
                                pks[tb][:], wt[:, kd * P:(kd + 1) * P],
                                xt[:, kd * T + tb * TQ: kd * T + (tb + 1) * TQ],
                                start=(kd == 0), stop=(kd == KD - 1))
                    for tb in range(NTB):
                        nc.any.tensor_copy(
                            kt_sb[:, do * T + tb * TQ: do * T + (tb + 1) * TQ],
                            pks[tb][:])

                # C: V natural, into 65-wide head slots
                for nh in range(2):
                    wvt = wp.tile([P, KD * TQ], F16, tag="wv", bufs=1)
                    nc.sync.dma_start(
                        wvt.rearrange("p (ko c) -> p ko c", c=TQ),
                        wq3[:, :, 2 * D + nh * TQ: 2 * D + (nh + 1) * TQ])
                    for tci in range(NT):
                        pv = pjps.tile([P, TQ], F32, tag="pj")
                        for kd in range(KD):
                            nc.tensor.matmul(
                                pv[:],
                                xt[:, kd * T + tci * P: kd * T + (tci + 1) * P],
                                wvt[:, kd * TQ:(kd + 1) * TQ],
                                start=(kd == 0), stop=(kd == KD - 1))
                        dst = v_sb[:, tci * (H * VW) + nh * 8 * VW:
                                   tci * (H * VW) + (nh + 1) * 8 * VW]
                        nc.any.tensor_copy(
                            dst.rearrange("p (h c) -> p h c", c=VW)[:, :, 0:DH],
                            pv.rearrange("p (h c) -> p h c", c=DH))

            # ---------------- phase E: attention + F: output proj ----------
            with (
                tc.tile_pool(name="otp", bufs=1) as otp,
                tc.tile_pool(name="ptp", bufs=6) as ptp,
                tc.tile_pool(name="rcp", bufs=2) as rcp,
                tc.tile_pool(name="rbp", bufs=3) as rbp,
                tc.tile_pool(name="wop", bufs=16) as wop,
            ):
                ot_sb = otp.tile([P, NPAIR * TQ], F16)      # 8 KB/part
                # preload Wo so phase F never waits on DMA
                wot = {}
                for ph in range(NPAIR):
                    for nh in range(2):
                        wot[ph, nh] = wop.tile([P, TQ], F16, tag="wo",
                                               name=f"wo_{ph}_{nh}")
                        nc.sync.dma_start(
                            wot[ph, nh][:],
                            wo[ph * P:(ph + 1) * P, nh * TQ:(nh + 1) * TQ])
                # logits/exp run in groups of GRP 512-wide half-chunks
                # ([128, 1536] 3-bank psum tiles -> one big ACTIVATE each);
                # PV of a chunk is emitted one group later (pipeline lag).
                GRP = 3
                attn_ps = tc.tile_pool(name="lgps", bufs=2, space="PSUM")
                lgps = attn_ps.__enter__()
                pv_ps = tc.tile_pool(name="pvps", bufs=2, space="PSUM")
                pvps = pv_ps.__enter__()
                for p in range(NPAIR):
                    kt = kt_sb[:, p * T:(p + 1) * T]
                    qh = (qt_sb[0:DH, p * TQ:(p + 1) * TQ],
                          qt_sb[DH:P, p * TQ:(p + 1) * TQ])
                    pva = pvps.tile([VW, TQ], F32, tag="pv")
                    pvb = pvps.tile([VW, TQ], F32, tag="pv")
                    halves = [(kc, hh) for kc in range(NT) for hh in (0, 1)]
                    groups = [halves[i:i + GRP] for i in range(0, len(halves), GRP)]
                    loc = {}      # (kc, hh) -> (pt tile, slot)
                    emitted = set()

                    def emit_pv_ready(done_through):
                        for kc in range(NT):
                            if kc in emitted:
                                continue
                            if (kc, 0) not in done_through or (kc, 1) not in done_through:
                                return
                            emitted.add(kc)
                            for hh, pv_ in ((0, pva), (1, pvb)):
                                h = 2 * p + hh
                                va = v_sb[:, kc * (H * VW) + h * VW:
                                          kc * (H * VW) + h * VW + VW]
                                pt_, j = loc[kc, hh]
                                nc.tensor.matmul(pv_[:], va,
                                                 pt_[:, j * TQ:(j + 1) * TQ],
                                                 start=(kc == 0),
                                                 stop=(kc == NT - 1))

                    done = set()
                    prev_done = set()
                    for g, grp in enumerate(groups):
                        # emit pv for chunks fully exp'd as of the PREVIOUS
                        # group first: keeps the PE fed while ACT works
                        emit_pv_ready(prev_done)
                        n = len(grp)
                        lg = lgps.tile([P, GRP * TQ], F32, tag="lg")
                        for j, (kc, hh) in enumerate(grp):
                            nc.tensor.matmul(
                                lg[:, j * TQ:(j + 1) * TQ],
                                kt[hh * DH:(hh + 1) * DH, kc * P:(kc + 1) * P],
                                qh[hh], start=True, stop=True)
                        pt_ = ptp.tile([P, GRP * TQ], F16, tag="pt")
                        nc.scalar.activation(pt_[:, 0:n * TQ], lg[:, 0:n * TQ],
                                             EXPF, scale=0.125)
                        for j, half in enumerate(grp):
                            loc[half] = (pt_, j)
                        prev_done = set(done)
                        done.update(grp)
                    emit_pv_ready(done)

                    # normalize: outT_h[dh, q] *= 1 / s[q].  Copy psum out
                    # first so the PV banks free fast for the next pair.
                    for hi, pv_ in ((0, pva), (1, pvb)):
                        pvs = rcp.tile([VW, TQ], F32, tag="pvs")
                        nc.vector.tensor_copy(pvs[:], pv_[:])
                        rc = rcp.tile([P, TQ], F32, tag="rc")
                        nc.vector.reciprocal(rc[DH:DH + 1, :], pvs[DH:DH + 1, :])
                        rb = rbp.tile([P, TQ], F32, tag="rb")
                        nc.sync.dma_start(
                            rb[0:DH, :],
                            rc[DH:DH + 1, :].unsqueeze(1)
                              .broadcast_to((1, DH, TQ)))
                        if hi == 0:
                            nc.vector.tensor_mul(
                                ot_sb[0:DH, p * TQ:(p + 1) * TQ],
                                pvs[0:DH, :], rb[0:DH, :])
                        else:
                            # head b lands at partitions 64-127 of ot_sb, but
                            # DVE cannot shift partitions: normalize into a
                            # staging tile then DMA-shift partitions.
                            sh = rbp.tile([P, TQ], F16, tag="sh")
                            nc.vector.tensor_mul(
                                sh[0:DH, :], pvs[0:DH, :], rb[0:DH, :])
                            nc.sync.dma_start(
                                ot_sb[DH:P, p * TQ:(p + 1) * TQ], sh[0:DH, :])
                pv_ps.__exit__(None, None, None)
                attn_ps.__exit__(None, None, None)

                # F: y = outT^T @ Wo
                with tc.tile_pool(name="fps", bufs=4, space="PSUM") as fps:
                  for qc in range(NQ):
                    pys = [fps.tile([P, TQ], F32, tag="f", name=f"py{qc}_{_n}")
                           for _n in range(2)]
                    for ph in range(NPAIR):
                        for nh in range(2):
                            nc.tensor.matmul(
                                pys[nh][:],
                                ot_sb[:, ph * TQ + qc * P: ph * TQ + (qc + 1) * P],
                                wot[ph, nh][:],
                                start=(ph == 0), stop=(ph == NPAIR - 1))
                    for nh in range(2):
                        ys = rbp.tile([P, TQ], F32, tag="rb")
                        nc.any.tensor_copy(ys[:], pys[nh][:])
                        nc.sync.dma_start(
                            y[qc * P:(qc + 1) * P, nh * TQ:(nh + 1) * TQ],
                            ys[:])
    nc.compile()
    return nc


_NC_CACHE = None


def _get_nc():
    global _NC_CACHE
    if _NC_CACHE is None:
        _NC_CACHE = build_nc()
    return _NC_CACHE


def _shard_inputs(x, Wqkv, Wo):
    x16 = np.asarray(x, dtype=np.float32).astype(np.float16)
    w16 = np.ascontiguousarray(np.asarray(Wqkv, dtype=np.float32).astype(np.float16))
    wo16 = np.ascontiguousarray(np.asarray(Wo, dtype=np.float32).astype(np.float16))
    in_maps = []
    for c in range(N_CORES):
        b, tb = c // NTB, c % NTB
        in_maps.append({
            "xb": np.ascontiguousarray(x16[b]),
            "xq": np.ascontiguousarray(x16[b, tb * TQ:(tb + 1) * TQ, :]),
            "wqkv": w16,
            "wo": wo16,
        })
    return in_maps


def kernel(x, Wqkv, Wo):
    from concourse.bass_utils import run_bass_kernel_spmd

    nc = _get_nc()
    in_maps = _shard_inputs(x, Wqkv, Wo)
    res = run_bass_kernel_spmd(nc, in_maps, core_ids=list(range(N_CORES)))
    out = np.empty((B, T, D), dtype=np.float32)
    for c in range(N_CORES):
        b, tb = c // NTB, c % NTB
        out[b, tb * TQ:(tb + 1) * TQ, :] = res.results[c]["y"]
    return out


# revision 27
# speedup vs baseline: 1.0773x; 1.0773x over previous
"""Multi-head attention (B=2, T=2048, D=1024, H=16, no causal mask) on 8 trn2
NeuronCores.

Sharding: pure data-parallel over (batch, query-token-block).  Core c handles
batch b = c // 4 and query rows [tb*512, (tb+1)*512) with tb = c % 4.  Each
core redundantly computes K and V for its whole batch (15.1 GFLOP/core vs 8.6
for tensor-parallel-heads) but needs NO collectives; an on-chip 4-rank 8 MB
AllReduce costs more than the redundant compute.

Precision: x/Wqkv and the whole QKV+attention pipeline run in fp16 (PE at
1 cyc/row with fp32 PSUM accumulation; fp32r measures ~2 cyc/row on HW even
warm; fp16 keeps 3 more mantissa bits than bf16 -> ~6e-4 final rel err vs
5e-3).  The output projection — whose input rounding feeds the result
directly — stays in float32r.

Per-core plan:
  1. PE-transpose X[b] (bf16) into XT (1024x2048 SBUF) and the query slice
     Xq into XqT.
  2. QT[do,:] = Wq[:,do]^T @ XqT       (QT:  [1024, 512]  bf16 SBUF)
     KT[do,:] = Wk[:,do]^T @ XT        (KT:  [1024, 2048] bf16 SBUF)
     V [tc,:] = XT[:,tc]^T @ Wv        (V:   [2048, 1024] bf16 SBUF, 65-wide
                                        head slots with a ones column ->
                                        PV also yields the softmax sum)
  3. Attention per head pair p (heads 2p, 2p+1 at partitions 0-63 / 64-127
     of KT/QT row chunk p):
       logitsT[k,q] = KT_h[:,kc]^T @ QT_h      (PSUM [128, 512] fp32)
       PT = exp(0.125 * logitsT)               (ScalarE, PSUM -> bf16 SBUF)
       outT_h[dh,q], s[q] = [V_h | 1]^T @ PT   (PSUM [65, 512], accum 16 kc)
     normalize: outT_h *= (1/s) broadcast across partitions via DMA,
     written to ot_sb as float32r.
  4. y[q,:] = outT^T @ Wo in float32r (accumulate 8 row chunks).
"""

import numpy as np

import concourse.bacc as bacc
import concourse.mybir as mybir
import concourse.tile as tile
from concourse.masks import make_identity

F32 = mybir.dt.float32
F32R = mybir.dt.float32r
F16 = mybir.dt.float16

B, T, D, H = 2, 2048, 1024, 16
DH = D // H  # 64
TQ = 512     # query tokens per core
N_CORES = 8
P = 128
KD = D // P        # 8 contraction chunks over D
NT = T // P        # 16 key-token chunks
NTB = T // TQ      # 4 token blocks
NPAIR = H // 2     # 8 head pairs
VW = DH + 1        # 65: V head slot width incl. ones column
NQ = TQ // P       # 4 query-token chunks
EXPF = mybir.ActivationFunctionType.Exp


def build_nc():
    nc = bacc.Bacc("TRN2", target_bir_lowering=False, debug=False,
                   num_devices=N_CORES)
    xb = nc.dram_tensor("xb", [T, D], F16, kind="ExternalInput").ap()
    xq = nc.dram_tensor("xq", [TQ, D], F16, kind="ExternalInput").ap()
    wqkv = nc.dram_tensor("wqkv", [D, 3 * D], F16, kind="ExternalInput").ap()
    wo = nc.dram_tensor("wo", [D, D], F16, kind="ExternalInput").ap()
    y = nc.dram_tensor("y", [TQ, D], F32, kind="ExternalOutput").ap()

    with tile.TileContext(nc) as tc:
        with tc.tile_pool(name="persist", bufs=1) as persist:
            v_sb = persist.tile([P, NT * H * VW], F16)    # 32.5 KB/part
            qt_sb = persist.tile([P, NPAIR * TQ], F16)    # 8 KB/part
            kt_sb = persist.tile([P, KD * T], F16)        # 32 KB/part
            ident = persist.tile([P, P], F16)
            make_identity(nc, ident)
            # ones columns in every (tok-chunk, head) V slot
            onec = persist.tile([P, 1], F16)
            nc.vector.memset(onec[:], 1.0)
            nc.vector.tensor_copy(
                v_sb.rearrange("p (b c) -> p b c", c=VW)[:, :, DH:DH + 1],
                onec.unsqueeze(1).broadcast_to((P, NT * H, 1)))

            # ---------------- phase A-D: transposes + projections ----------
            with (
                tc.tile_pool(name="xtp", bufs=1) as xtp,
                tc.tile_pool(name="xin", bufs=3) as xinp,
                tc.tile_pool(name="wp", bufs=1) as wp,
                tc.tile_pool(name="trps", bufs=3, space="PSUM") as trps,
                tc.tile_pool(name="pjps", bufs=5, space="PSUM") as pjps,
            ):
                xt = xtp.tile([P, KD * T], F16)    # 32 KB/part
                xqt = xtp.tile([P, KD * TQ], F16)  # 8 KB/part

                # A: PE-transpose xq then xb (xq first so QT can start early;
                # a DMA-xbar transpose would serialize ~35us before any proj
                # matmul can start, since every proj MM contracts over all
                # of D).  8 kd-subtiles share one psum bank; one strided
                # copy scatters them into xt/xqt.
                def transpose_chunk(src_row, dst, dst_off, dst_stride):
                    ps = trps.tile([P, KD * P], F16, tag="tr")
                    for kd in range(KD):
                        nc.tensor.transpose(
                            ps[:, kd * P:(kd + 1) * P],
                            src_row[:, kd * P:(kd + 1) * P], ident[:])
                    nc.any.tensor_copy(
                        dst.rearrange("p (k c) -> p k c", c=dst_stride)
                           [:, :, dst_off:dst_off + P],
                        ps.rearrange("p (k c) -> p k c", c=P))

                for tci in range(NQ):
                    xin = xinp.tile([P, D], F16, tag="xin")
                    nc.sync.dma_start(xin[:], xq[tci * P:(tci + 1) * P, :])
                    transpose_chunk(xin, xqt, tci * P, TQ)
                for tci in range(NT):
                    xin = xinp.tile([P, D], F16, tag="xin")
                    nc.sync.dma_start(xin[:], xb[tci * P:(tci + 1) * P, :])
                    transpose_chunk(xin, xt, tci * P, T)

                # wqkv viewed as [p, ko, col] so a whole K-column strip of a
                # weight loads with one DMA
                wq3 = wqkv.rearrange("(ko p) c -> p ko c", p=P)

                # D: QT (dout chunk do covers heads 2do, 2do+1)
                for do in range(KD):
                    wt = wp.tile([P, KD * P], F16, tag="wk", bufs=2)
                    nc.sync.dma_start(
                        wt.rearrange("p (ko c) -> p ko c", c=P),
                        wq3[:, :, do * P:(do + 1) * P])
                    pq = pjps.tile([P, TQ], F32, tag="pj")
                    for kd in range(KD):
                        nc.tensor.matmul(
                            pq[:], wt[:, kd * P:(kd + 1) * P],
                            xqt[:, kd * TQ:(kd + 1) * TQ],
                            start=(kd == 0), stop=(kd == KD - 1))
                    nc.any.tensor_copy(qt_sb[:, do * TQ:(do + 1) * TQ], pq[:])

                # B: KT.  kd-outer / tb-inner: one LDWEIGHTS serves 4 matmuls
                # (4 open psum accumulators).
                for do in range(KD):
                    wt = wp.tile([P, KD * P], F16, tag="wk", bufs=2)
                    nc.sync.dma_start(
                        wt.rearrange("p (ko c) -> p ko c", c=P),
                        wq3[:, :, D + do * P: D + (do + 1) * P])
                    pks = [pjps.tile([P, TQ], F32, tag="pj", name=f"pk{do}_{_t}")
                           for _t in range(NTB)]
                    for kd in range(KD):
                        for tb in range(NTB):
                            nc.tensor.matmul(
                                pks[tb][:], wt[:, kd * P:(kd + 1) * P],
                                xt[:, kd * T + tb * TQ: kd * T + (tb + 1) * TQ],
                                start=(kd == 0), stop=(kd == KD - 1))
                    for tb in range(NTB):
                        nc.any.tensor_copy(
                            kt_sb[:, do * T + tb * TQ: do * T + (tb + 1) * TQ],
                            pks[tb][:])

                # C: V natural, into 65-wide head slots
                for nh in range(2):
                    wvt = wp.tile([P, KD * TQ], F16, tag="wv", bufs=1)
                    nc.sync.dma_start(
                        wvt.rearrange("p (ko c) -> p ko c", c=TQ),
                        wq3[:, :, 2 * D + nh * TQ: 2 * D + (nh + 1) * TQ])
                    for tci in range(NT):
                        pv = pjps.tile([P, TQ], F32, tag="pj")
                        for kd in range(KD):
                            nc.tensor.matmul(
                                pv[:],
                                xt[:, kd * T + tci * P: kd * T + (tci + 1) * P],
                                wvt[:, kd * TQ:(kd + 1) * TQ],
                                start=(kd == 0), stop=(kd == KD - 1))
                        dst = v_sb[:, tci * (H * VW) + nh * 8 * VW:
                                   tci * (H * VW) + (nh + 1) * 8 * VW]
                        nc.any.tensor_copy(
                            dst.rearrange("p (h c) -> p h c", c=VW)[:, :, 0:DH],
                            pv.rearrange("p (h c) -> p h c", c=DH))

            # ---------------- phase E: attention + F: output proj ----------
            with (
                tc.tile_pool(name="otp", bufs=1) as otp,
                tc.tile_pool(name="ptp", bufs=6) as ptp,
                tc.tile_pool(name="rcp", bufs=2) as rcp,
                tc.tile_pool(name="rbp", bufs=3) as rbp,
                tc.tile_pool(name="wop", bufs=16) as wop,
            ):
                ot_sb = otp.tile([P, NPAIR * TQ], F16)      # 8 KB/part
                # preload Wo so phase F never waits on DMA
                wot = {}
                for ph in range(NPAIR):
                    for nh in range(2):
                        wot[ph, nh] = wop.tile([P, TQ], F16, tag="wo",
                                               name=f"wo_{ph}_{nh}")
                        nc.sync.dma_start(
                            wot[ph, nh][:],
                            wo[ph * P:(ph + 1) * P, nh * TQ:(nh + 1) * TQ])
                # logits/exp run in groups of GRP 512-wide half-chunks
                # ([128, 1536] 3-bank psum tiles -> one big ACTIVATE each);
                # PV of a chunk is emitted one group later (pipeline lag).
                GRP = 3
                attn_ps = tc.tile_pool(name="lgps", bufs=2, space="PSUM")
                lgps = attn_ps.__enter__()
                pv_ps = tc.tile_pool(name="pvps", bufs=2, space="PSUM")
                pvps = pv_ps.__enter__()
                for p in range(NPAIR):
                    kt = kt_sb[:, p * T:(p + 1) * T]
                    qh = (qt_sb[0:DH, p * TQ:(p + 1) * TQ],
                          qt_sb[DH:P, p * TQ:(p + 1) * TQ])
                    pva = pvps.tile([VW, TQ], F32, tag="pv")
                    pvb = pvps.tile([VW, TQ], F32, tag="pv")
                    halves = [(kc, hh) for kc in range(NT) for hh in (0, 1)]
                    groups = [halves[i:i + GRP] for i in range(0, len(halves), GRP)]
                    loc = {}      # (kc, hh) -> (pt tile, slot)
                    emitted = set()

                    def emit_pv_ready(done_through):
                        for kc in range(NT):
                            if kc in emitted:
                                continue
                            if (kc, 0) not in done_through or (kc, 1) not in done_through:
                                return
                            emitted.add(kc)
                            for hh, pv_ in ((0, pva), (1, pvb)):
                                h = 2 * p + hh
                                va = v_sb[:, kc * (H * VW) + h * VW:
                                          kc * (H * VW) + h * VW + VW]
                                pt_, j = loc[kc, hh]
                                nc.tensor.matmul(pv_[:], va,
                                                 pt_[:, j * TQ:(j + 1) * TQ],
                                                 start=(kc == 0),
                                                 stop=(kc == NT - 1))

                    done = set()
                    prev_done = set()
                    for g, grp in enumerate(groups):
                        # emit pv for chunks fully exp'd as of the PREVIOUS
                        # group first: keeps the PE fed while ACT works
                        emit_pv_ready(prev_done)
                        n = len(grp)
                        lg = lgps.tile([P, GRP * TQ], F32, tag="lg")
                        for j, (kc, hh) in enumerate(grp):
                            nc.tensor.matmul(
                                lg[:, j * TQ:(j + 1) * TQ],
                                kt[hh * DH:(hh + 1) * DH, kc * P:(kc + 1) * P],
                                qh[hh], start=True, stop=True)
                        pt_ = ptp.tile([P, GRP * TQ], F16, tag="pt")
                        nc.scalar.activation(pt_[:, 0:n * TQ], lg[:, 0:n * TQ],
                                             EXPF, scale=0.125)
                        for j, half in enumerate(grp):
                            loc[half] = (pt_, j)
                        prev_done = set(done)
                        done.update(grp)
                    emit_pv_ready(done)

                    # normalize: outT_h[dh, q] *= 1 / s[q].  Copy psum out
                    # first so the PV banks free fast for the next pair.
                    for hi, pv_ in ((0, pva), (1, pvb)):
                        pvs = rcp.tile([VW, TQ], F32, tag="pvs")
                        nc.vector.tensor_copy(pvs[:], pv_[:])
                        rc = rcp.tile([P, TQ], F32, tag="rc")
                        nc.vector.reciprocal(rc[DH:DH + 1, :], pvs[DH:DH + 1, :])
                        rb = rbp.tile([P, TQ], F32, tag="rb")
                        nc.sync.dma_start(
                            rb[0:DH, :],
                            rc[DH:DH + 1, :].unsqueeze(1)
                              .broadcast_to((1, DH, TQ)))
                        if hi == 0:
                            nc.vector.tensor_mul(
                                ot_sb[0:DH, p * TQ:(p + 1) * TQ],
                                pvs[0:DH, :], rb[0:DH, :])
                        else:
                            # head b lands at partitions 64-127 of ot_sb, but
                            # DVE cannot shift partitions: normalize into a
                            # staging tile then DMA-shift partitions.
                            sh = rbp.tile([P, TQ], F16, tag="sh")
                            nc.vector.tensor_mul(
                                sh[0:DH, :], pvs[0:DH, :], rb[0:DH, :])
                            nc.sync.dma_start(
                                ot_sb[DH:P, p * TQ:(p + 1) * TQ], sh[0:DH, :])
                pv_ps.__exit__(None, None, None)
                attn_ps.__exit__(None, None, None)

                # F: y = outT^T @ Wo
                with tc.tile_pool(name="fps", bufs=4, space="PSUM") as fps:
                  for qc in range(NQ):
                    pys = [fps.tile([P, TQ], F32, tag="f", name=f"py{qc}_{_n}")
                           for _n in range(2)]
                    for ph in range(NPAIR):
                        for nh in range(2):
                            nc.tensor.matmul(
                                pys[nh][:],
                                ot_sb[:, ph * TQ + qc * P: ph * TQ + (qc + 1) * P],
                                wot[ph, nh][:],
                                start=(ph == 0), stop=(ph == NPAIR - 1))
                    for nh in range(2):
                        ys = rbp.tile([P, TQ], F32, tag="rb")
                        nc.any.tensor_copy(ys[:], pys[nh][:])
                        nc.sync.dma_start(
                            y[qc * P:(qc + 1) * P, nh * TQ:(nh + 1) * TQ],
                            ys[:])
    nc.compile()
    return nc


_NC_CACHE = None


def _get_nc():
    global _NC_CACHE
    if _NC_CACHE is None:
        _NC_CACHE = build_nc()
    return _NC_CACHE


def _shard_inputs(x, Wqkv, Wo):
    x16 = np.asarray(x, dtype=np.float32).astype(np.float16)
    w16 = np.ascontiguousarray(np.asarray(Wqkv, dtype=np.float32).astype(np.float16))
    wo16 = np.ascontiguousarray(np.asarray(Wo, dtype=np.float32).astype(np.float16))
    in_maps = []
    for c in range(N_CORES):
        b, tb = c // NTB, c % NTB
        in_maps.append({
            "xb": np.ascontiguousarray(x16[b]),
            "xq": np.ascontiguousarray(x16[b, tb * TQ:(tb + 1) * TQ, :]),
            "wqkv": w16,
            "wo": wo16,
        })
    return in_maps


def kernel(x, Wqkv, Wo):
    from concourse.bass_utils import run_bass_kernel_spmd

    nc = _get_nc()
    in_maps = _shard_inputs(x, Wqkv, Wo)
    res = run_bass_kernel_spmd(nc, in_maps, core_ids=list(range(N_CORES)))
    out = np.empty((B, T, D), dtype=np.float32)
    for c in range(N_CORES):
        b, tb = c // NTB, c % NTB
        out[b, tb * TQ:(tb + 1) * TQ, :] = res.results[c]["y"]
    return out
